# revision 7
# baseline (speedup 1.0000x reference)
"""Trainium2 Bass kernel for nn_ActivationAndBiophysModel.

2-layer GRU (H=512) + FC + antagonist-muscle biophysics, T=512 steps, B=64.

v2: fp8-e4m3 DoubleRow matmuls. The recurrence is PE-column-streaming
bound; DoubleRow fp8 streams 2 weight columns/cycle and contracts K=256
per pass (vs bf16 1 col/cycle, K=128). Precision scheme (validated in
sim, rel_err 4.9e-3 vs 2e-2 gate):
  - r/z gates: single pass  W_hi(x512) . h_hi(x16)        [S = 8192]
  - n gates + FC: 3 passes  W_hi.h_hi + W_ds(x16).h_lo(x512) + W_lo.h_hi
  - x path + all biases: bf16 plain matmuls (exact); b_hh*n biases ride
    the PSUM-descale scalar_tensor_tensor op.
  - h carried bf16; h_hi/h_lo fp8 stationaries rebuilt per step via PE
    transpose + convert ops.
"""

import sys

for p in ("/opt/trn_rl_repo", "/opt/pypackages"):
    if p not in sys.path:
        sys.path.insert(0, p)

import numpy as np  # noqa: E402

B, T, IN, H, J = 64, 512, 16, 512, 8
HG = 3 * H

SW = 512.0      # W_hi scale
SH = 16.0       # h_hi scale
SL = 512.0      # h_lo scale
SDS = 16.0      # W_ds scale (pairs with h_lo: 16*512 = 8192)
S = SW * SH     # 8192 PSUM scale
C = 1.0 / S

# muscle / joint constants
K0, K1, L0m, L1m, Mm = 100.0, 2000.0, 0.06, 0.006, 0.05
Ij, Kj, Bj, DT = 0.004, 5.0, 0.3, 1.0 / 60.0
_c = DT / Ij
ALPHA = 1.0 - _c * Bj
BETA = _c * Mm * (K0 * L1m + K1 * L0m)
GAMMA = _c * Mm * K1 * L1m
DELTA = _c * (-(2.0 * Mm * Mm * K0) - Kj)
EPS = _c * (-(Mm * Mm * K1))


def _build(nc, bass, tile, mybir, T_run):
    f32 = mybir.dt.float32
    bf = mybir.dt.bfloat16
    f8 = mybir.dt.float8e4
    AF = mybir.ActivationFunctionType
    Alu = mybir.AluOpType
    DR = mybir.MatmulPerfMode.DoubleRow

    # ---- DRAM parameters -------------------------------------------------
    xT_d = nc.declare_dram_parameter("xT", [IN + 1, T_run, B], bf, isOutput=False)
    w0x_d = nc.declare_dram_parameter("w0x", [IN + 1, HG], bf, isOutput=False)
    whi_d = {}
    wds_d = {}
    wlo_d = {}
    for nm in ("w0h", "w1i", "w1h"):
        whi_d[nm] = nc.declare_dram_parameter(f"{nm}hi", [4, 128, HG], f8, isOutput=False)
        wds_d[nm] = nc.declare_dram_parameter(f"{nm}ds", [4, 128, H], f8, isOutput=False)
        wlo_d[nm] = nc.declare_dram_parameter(f"{nm}lo", [4, 128, H], f8, isOutput=False)
    wfchi_d = nc.declare_dram_parameter("wfchi", [4, 128, 2 * J], f8, isOutput=False)
    wfcds_d = nc.declare_dram_parameter("wfcds", [4, 128, 2 * J], f8, isOutput=False)
    wfclo_d = nc.declare_dram_parameter("wfclo", [4, 128, 2 * J], f8, isOutput=False)
    # b1rz*S (1024) | b_ih1n*S (512) | fc_b*S (16) | ones (B)
    brows_d = nc.declare_dram_parameter("brows", [1, 1552 + B], bf, isOutput=False)
    bhn_d = nc.declare_dram_parameter("bhn", [B, 1024], bf, isOutput=False)
    ident_d = nc.declare_dram_parameter("ident", [B, B], bf, isOutput=False)
    hb0_d = nc.declare_dram_parameter("hb0", [2, B, H], bf, isOutput=False)
    hT8hi0_d = nc.declare_dram_parameter("hT8hi0", [2, 128, 4, B], f8, isOutput=False)
    hT8lo0_d = nc.declare_dram_parameter("hT8lo0", [2, 128, 4, B], f8, isOutput=False)
    th0_d = nc.declare_dram_parameter("th0", [B, J], f32, isOutput=False)
    om0_d = nc.declare_dram_parameter("om0", [B, J], f32, isOutput=False)
    out_d = nc.declare_dram_parameter("out", [B, T_run * J], f32, isOutput=True)

    with tile.TileContext(nc) as tc:
        with (
            tc.tile_pool(name="wpool", bufs=1) as wp,
            tc.tile_pool(name="xpool", bufs=8) as xp,
            tc.tile_pool(name="state", bufs=2) as sp,
            tc.tile_pool(name="gates", bufs=2) as gp,
            tc.tile_pool(name="bp", bufs=2) as bp,
            tc.tile_pool(name="prz", bufs=2, space="PSUM") as prz,
            tc.tile_pool(name="psm", bufs=4, space="PSUM") as psm,
        ):
            # ---- load constants/weights once -----------------------------
            w0x = wp.tile([IN + 1, HG], bf)
            nc.sync.dma_start(w0x[:], w0x_d[:])
            whi = {}
            wds = {}
            wlo = {}
            for nm in ("w0h", "w1i", "w1h"):
                whi[nm] = wp.tile([128, 4, HG], f8, tag=f"whi{nm}", name=f"whi{nm}")
                wds[nm] = wp.tile([128, 4, H], f8, tag=f"wds{nm}", name=f"wds{nm}")
                wlo[nm] = wp.tile([128, 4, H], f8, tag=f"wlo{nm}", name=f"wlo{nm}")
                for c4 in range(4):
                    nc.sync.dma_start(whi[nm][:, c4, :], whi_d[nm][c4])
                    nc.sync.dma_start(wds[nm][:, c4, :], wds_d[nm][c4])
                    nc.sync.dma_start(wlo[nm][:, c4, :], wlo_d[nm][c4])
            wfchi = wp.tile([128, 4, 2 * J], f8)
            wfcds = wp.tile([128, 4, 2 * J], f8)
            wfclo = wp.tile([128, 4, 2 * J], f8)
            for c4 in range(4):
                nc.sync.dma_start(wfchi[:, c4, :], wfchi_d[c4])
                nc.sync.dma_start(wfcds[:, c4, :], wfcds_d[c4])
                nc.sync.dma_start(wfclo[:, c4, :], wfclo_d[c4])
            brows = wp.tile([1, 1552 + B], bf)
            nc.sync.dma_start(brows[:], brows_d[:])
            ones = brows[:, 1552 : 1552 + B]
            bhn = wp.tile([B, 1024], bf)
            nc.sync.dma_start(bhn[:], bhn_d[:])
            ident = wp.tile([B, B], bf)
            nc.sync.dma_start(ident[:], ident_d[:])
            out_sb = wp.tile([B, T_run * J], f32)

            # ---- initial state -------------------------------------------
            h0b = sp.tile([B, H], bf, tag="h0b")
            h1b = sp.tile([B, H], bf, tag="h1b")
            nc.sync.dma_start(h0b[:], hb0_d[0])
            nc.sync.dma_start(h1b[:], hb0_d[1])
            h0hi = sp.tile([128, 4, B], f8, tag="h0hi")
            h0lo = sp.tile([128, 4, B], f8, tag="h0lo")
            h1hi = sp.tile([128, 4, B], f8, tag="h1hi")
            h1lo = sp.tile([128, 4, B], f8, tag="h1lo")
            for c4 in range(4):
                nc.sync.dma_start(h0hi[:, c4, :], hT8hi0_d[0, :, c4, :])
                nc.sync.dma_start(h0lo[:, c4, :], hT8lo0_d[0, :, c4, :])
                nc.sync.dma_start(h1hi[:, c4, :], hT8hi0_d[1, :, c4, :])
                nc.sync.dma_start(h1lo[:, c4, :], hT8lo0_d[1, :, c4, :])
            th_init = sp.tile([B, J], f32, tag="th")
            nc.sync.dma_start(th_init[:], th0_d[:])
            om = sp.tile([B, J], f32, tag="om")
            nc.sync.dma_start(om[:], om0_d[:])
            th_ap = th_init[:]

            xts = [None] * (T_run + 2)

            def dma_x(t):
                if t < T_run and xts[t] is None:
                    xt = xp.tile([IN + 1, B], bf, tag="xt")
                    nc.sync.dma_start(xt[:], xT_d[:, t, :])
                    xts[t] = xt

            def mm(out, lhsT, rhs, start, stop):
                nc.tensor.matmul(out, lhsT, rhs, start=start, stop=stop,
                                 perf_mode=DR)

            def mmb(out, lhsT, rhs, start, stop):
                nc.tensor.matmul(out, lhsT, rhs, start=start, stop=stop)

            def l0_mms(rz, hn, inn, hhi, hlo, xt):
                w = whi["w0h"]
                # rz first (so sigmoid launches early), stationary-grouped
                for p in range(2):
                    st = hhi[:, 2 * p : 2 * p + 2, :]
                    for b in range(2):
                        mm(rz[:, b * 512 : b * 512 + 512], st,
                           w[:, 2 * p : 2 * p + 2, b * 512 : b * 512 + 512],
                           start=(p == 0), stop=False)
                    mm(hn[:], st, w[:, 2 * p : 2 * p + 2, 1024:1536],
                       start=(p == 0), stop=False)
                    mm(hn[:], st, wlo["w0h"][:, 2 * p : 2 * p + 2, :],
                       start=False, stop=False)
                for b in range(2):
                    mmb(rz[:, b * 512 : b * 512 + 512], xt[:],
                        w0x[:, b * 512 : b * 512 + 512], start=False, stop=True)
                for p in range(2):
                    mm(hn[:], hlo[:, 2 * p : 2 * p + 2, :],
                       wds["w0h"][:, 2 * p : 2 * p + 2, :],
                       start=False, stop=(p == 1))
                mmb(inn[:], xt[:], w0x[:, 1024:1536], start=True, stop=True)

            def l1_hh_mms(rz, hn, hhi, hlo):
                w = whi["w1h"]
                for p in range(2):
                    st = hhi[:, 2 * p : 2 * p + 2, :]
                    for b in range(2):
                        mm(rz[:, b * 512 : b * 512 + 512], st,
                           w[:, 2 * p : 2 * p + 2, b * 512 : b * 512 + 512],
                           start=(p == 0), stop=False)
                    mm(hn[:], st, w[:, 2 * p : 2 * p + 2, 1024:1536],
                       start=(p == 0), stop=False)
                    mm(hn[:], st, wlo["w1h"][:, 2 * p : 2 * p + 2, :],
                       start=False, stop=False)
                for p in range(2):
                    mm(hn[:], hlo[:, 2 * p : 2 * p + 2, :],
                       wds["w1h"][:, 2 * p : 2 * p + 2, :],
                       start=False, stop=(p == 1))

            def l1_ih_mms(rz, inn, hhi, hlo):
                w = whi["w1i"]
                for p in range(2):
                    st = hhi[:, 2 * p : 2 * p + 2, :]
                    for b in range(2):
                        mm(rz[:, b * 512 : b * 512 + 512], st,
                           w[:, 2 * p : 2 * p + 2, b * 512 : b * 512 + 512],
                           start=False, stop=False)
                    mm(inn[:], st, w[:, 2 * p : 2 * p + 2, 1024:1536],
                       start=(p == 0), stop=False)
                    mm(inn[:], st, wlo["w1i"][:, 2 * p : 2 * p + 2, :],
                       start=False, stop=False)
                # bf16 bias rows close the rz group early
                for b in range(2):
                    mmb(rz[:, b * 512 : b * 512 + 512], ones,
                        brows[:, b * 512 : b * 512 + 512], start=False, stop=True)
                for p in range(2):
                    mm(inn[:], hlo[:, 2 * p : 2 * p + 2, :],
                       wds["w1i"][:, 2 * p : 2 * p + 2, :],
                       start=False, stop=False)
                mmb(inn[:], ones, brows[:, 1024:1536], start=False, stop=True)

            def fc_mms(fc, hhi, hlo):
                for p in range(2):
                    st = hhi[:, 2 * p : 2 * p + 2, :]
                    mm(fc, st, wfchi[:, 2 * p : 2 * p + 2, :],
                       start=(p == 0), stop=False)
                    mm(fc, st, wfclo[:, 2 * p : 2 * p + 2, :],
                       start=False, stop=False)
                for p in range(2):
                    mm(fc, hlo[:, 2 * p : 2 * p + 2, :],
                       wds_fc_ap[:, 2 * p : 2 * p + 2, :],
                       start=False, stop=False)
                mmb(fc, ones, brows[:, 1536:1552], start=False, stop=True)

            wds_fc_ap = wfcds

            def gru_vec(rz, hn, inn, lo, hb_prev, hb_tag):
                rzs = gp.tile([B, 1024], bf, tag=f"rzs{lo}")
                nc.scalar.activation(rzs[:], rz[:], AF.Sigmoid, scale=C)
                r_ = rzs[:, 0:512]
                z_ = rzs[:, 512:1024]
                hnb = gp.tile([B, H], bf, tag=f"hnb{lo}")
                nc.vector.scalar_tensor_tensor(
                    hnb[:], hn[:], C, bhn[:, lo * 512 : lo * 512 + 512],
                    Alu.mult, Alu.add)
                t_ = gp.tile([B, H], bf, tag=f"t{lo}")
                nc.vector.tensor_mul(t_[:], r_, hnb[:])
                u_ = gp.tile([B, H], bf, tag=f"u{lo}")
                nc.vector.scalar_tensor_tensor(
                    u_[:], inn[:], C, t_[:], Alu.mult, Alu.add)
                n_ = gp.tile([B, H], bf, tag=f"n{lo}")
                nc.scalar.activation(n_[:], u_[:], AF.Tanh)
                d_ = gp.tile([B, H], bf, tag=f"d{lo}")
                nc.vector.tensor_sub(d_[:], hb_prev[:], n_[:])
                m_ = gp.tile([B, H], bf, tag=f"m{lo}")
                nc.vector.tensor_mul(m_[:], z_, d_[:])
                hb = sp.tile([B, H], bf, tag=hb_tag)
                nc.vector.tensor_add(hb[:], m_[:], n_[:])
                return hb

            def transpose_convert(hb, hi_tag, lo_tag, lo):
                ph_a = psm.tile([128, 2, B], bf, tag="sm")
                ph_b = psm.tile([128, 2, B], bf, tag="sm")
                hT = gp.tile([128, 4, B], bf, tag=f"hT{lo}")
                for c4 in range(2):
                    nc.tensor.transpose(ph_a[:, c4, :],
                                        hb[:, c4 * 128 : (c4 + 1) * 128], ident[:])
                nc.vector.tensor_copy(hT[:, 0:2, :], ph_a[:])
                for c4 in range(2, 4):
                    nc.tensor.transpose(ph_b[:, c4 - 2, :],
                                        hb[:, c4 * 128 : (c4 + 1) * 128], ident[:])
                nc.scalar.activation(hT[:, 2:4, :], ph_b[:], AF.Copy)
                hi = sp.tile([128, 4, B], f8, tag=hi_tag)
                nc.scalar.activation(hi[:], hT[:], AF.Copy, scale=SH)
                e_ = gp.tile([128, 4, B], bf, tag=f"e{lo}")
                nc.vector.scalar_tensor_tensor(
                    e_[:], hi[:], -1.0 / SH, hT[:], Alu.mult, Alu.add)
                lo8 = sp.tile([128, 4, B], f8, tag=lo_tag)
                nc.scalar.activation(lo8[:], e_[:], AF.Copy, scale=SL)
                return hi, lo8

            # ---- prologue ------------------------------------------------
            dma_x(0)
            dma_x(1)
            rz0 = prz.tile([B, 1024], f32, tag="rz")
            hn0 = psm.tile([B, 512], f32, tag="sm")
            inn0 = psm.tile([B, 512], f32, tag="sm")
            l0_mms(rz0, hn0, inn0, h0hi, h0lo, xts[0])
            rz1 = prz.tile([B, 1024], f32, tag="rz")
            hn1 = psm.tile([B, 512], f32, tag="sm")
            l1_hh_mms(rz1, hn1, h1hi, h1lo)

            # ---- time loop -----------------------------------------------
            for t in range(T_run):
                dma_x(t + 2)
                last = t + 1 >= T_run

                h0b = gru_vec(rz0, hn0, inn0, 0, h0b, "h0b")
                h0hi, h0lo = transpose_convert(h0b, "h0hi", "h0lo", 0)

                inn1 = psm.tile([B, 512], f32, tag="sm")
                l1_ih_mms(rz1, inn1, h0hi, h0lo)
                if not last:
                    rz0_n = prz.tile([B, 1024], f32, tag="rz")
                    hn0 = psm.tile([B, 512], f32, tag="sm")
                    inn0 = psm.tile([B, 512], f32, tag="sm")
                    l0_mms(rz0_n, hn0, inn0, h0hi, h0lo, xts[t + 1])

                h1b = gru_vec(rz1, hn1, inn1, 1, h1b, "h1b")
                h1hi, h1lo = transpose_convert(h1b, "h1hi", "h1lo", 1)

                ps_fc = psm.tile([B, 2 * J], f32, tag="sm")
                fc_mms(ps_fc[:, :], h1hi, h1lo)
                if not last:
                    rz1_n = prz.tile([B, 1024], f32, tag="rz")
                    hn1 = psm.tile([B, 512], f32, tag="sm")
                    l1_hh_mms(rz1_n, hn1, h1hi, h1lo)
                    rz0, rz1 = rz0_n, rz1_n

                # fc sigmoid + biophysics
                a_s = bp.tile([B, J, 2], f32, tag="as")
                nc.scalar.activation(a_s[:], ps_fc[:, :], AF.Sigmoid, scale=C)
                a0 = a_s[:, :, 0]
                a1 = a_s[:, :, 1]
                s_ = bp.tile([B, J], f32, tag="s")
                nc.gpsimd.tensor_add(s_[:], a1, a0)
                dd = bp.tile([B, J], f32, tag="dd")
                nc.gpsimd.tensor_sub(dd[:], a1, a0)
                p_ = bp.tile([B, J], f32, tag="p")
                nc.gpsimd.tensor_mul(p_[:], s_[:], dd[:])
                w_ = bp.tile([B, J], f32, tag="w")
                nc.vector.scalar_tensor_tensor(
                    w_[:], dd[:], BETA / GAMMA, p_[:], Alu.mult, Alu.add)
                v_ = bp.tile([B, J], f32, tag="v")
                nc.vector.tensor_scalar(v_[:], s_[:], EPS, DELTA,
                                        Alu.mult, Alu.add)
                u2 = bp.tile([B, J], f32, tag="u2")
                nc.vector.tensor_mul(u2[:], v_[:], th_ap)
                q_ = bp.tile([B, J], f32, tag="q")
                nc.vector.scalar_tensor_tensor(
                    q_[:], w_[:], GAMMA, u2[:], Alu.mult, Alu.add)
                om_new = sp.tile([B, J], f32, tag="om")
                nc.vector.scalar_tensor_tensor(
                    om_new[:], om[:], ALPHA, q_[:], Alu.mult, Alu.add)
                om = om_new
                th_new = out_sb[:, t * J : (t + 1) * J]
                nc.vector.scalar_tensor_tensor(
                    th_new, om[:], DT, th_ap, Alu.mult, Alu.add)
                th_ap = th_new

            nc.sync.dma_start(out_d[:], out_sb[:])
    return nc


_NC_CACHE = {}


def _get_nc(T_run):
    if T_run in _NC_CACHE:
        return _NC_CACHE[T_run]
    from concourse import bass, bacc, tile

    mybir = bass.mybir
    nc = bacc.Bacc(None, target_bir_lowering=False)
    _build(nc, bass, tile, mybir, T_run)
    nc.compile()
    _NC_CACHE[T_run] = nc
    return nc


def _prep_inputs(x, W_ih0, W_hh0, b_ih0, b_hh0, W_ih1, W_hh1, b_ih1, b_hh1,
                 fc_W, fc_b, h0, theta0, omega0):
    import ml_dtypes

    F8 = ml_dtypes.float8_e4m3
    BF = ml_dtypes.bfloat16
    T_run = x.shape[1]
    f = np.float32

    def f8q(a, s):
        return np.clip(np.asarray(a, f) * s, -224, 224).astype(F8)

    out = {}

    xT = np.concatenate(
        [np.ascontiguousarray(x.transpose(2, 1, 0)),
         np.ones((1, T_run, B), f)], axis=0).astype(f)
    out["xT"] = xT.astype(BF)

    # layer-0 bias row rides w0x (bf16, pre-scaled by S)
    b0rz = (b_ih0 + b_hh0)[:1024]
    w0x = np.concatenate(
        [W_ih0.T, np.concatenate([b0rz, b_ih0[1024:]])[None, :]], axis=0
    ).astype(f)
    out["w0x"] = (w0x * S).astype(BF)

    for nm, W in (("w0h", W_hh0), ("w1i", W_ih1), ("w1h", W_hh1)):
        WT = np.ascontiguousarray(W.T).astype(f)        # [512, 1536]
        hi = f8q(WT, SW)
        res = WT * SW - hi.astype(f)                     # exact residual * SW
        lo = np.clip(res[:, 1024:], -224, 224).astype(F8)
        ds = f8q(WT[:, 1024:], SDS)
        out[f"{nm}hi"] = np.ascontiguousarray(hi.reshape(4, 128, HG))
        out[f"{nm}lo"] = np.ascontiguousarray(lo.reshape(4, 128, H))
        out[f"{nm}ds"] = np.ascontiguousarray(ds.reshape(4, 128, H))

    WfcT = np.ascontiguousarray(fc_W.T).astype(f)        # [512, 16]
    fchi = f8q(WfcT, SW)
    fcres = WfcT * SW - fchi.astype(f)
    out["wfchi"] = np.ascontiguousarray(fchi.reshape(4, 128, 2 * J))
    out["wfclo"] = np.ascontiguousarray(
        np.clip(fcres, -224, 224).astype(F8).reshape(4, 128, 2 * J))
    out["wfcds"] = np.ascontiguousarray(f8q(WfcT, SDS).reshape(4, 128, 2 * J))

    brows = np.zeros((1, 1552 + B), f)
    brows[0, 0:1024] = (b_ih1 + b_hh1)[:1024] * S
    brows[0, 1024:1536] = b_ih1[1024:] * S
    brows[0, 1536:1552] = fc_b * S
    brows[0, 1552:] = 1.0
    out["brows"] = brows.astype(BF)

    bhn = np.concatenate([np.tile(b_hh0[1024:][None, :], (B, 1)),
                          np.tile(b_hh1[1024:][None, :], (B, 1))],
                         axis=1).astype(f)
    out["bhn"] = bhn.astype(BF)
    out["ident"] = np.eye(B, dtype=f).astype(BF)
    out["hb0"] = h0.astype(f).astype(BF)

    hT = np.stack([np.ascontiguousarray(h0[l].T).astype(f) for l in range(2)])
    hThi = f8q(hT, SH)                                   # [2, 512, 64]
    hres = hT - hThi.astype(f) / SH
    hTlo = f8q(hres, SL)
    out["hT8hi0"] = np.ascontiguousarray(
        hThi.reshape(2, 4, 128, B).transpose(0, 2, 1, 3))
    out["hT8lo0"] = np.ascontiguousarray(
        hTlo.reshape(2, 4, 128, B).transpose(0, 2, 1, 3))
    out["th0"] = theta0.astype(f)
    out["om0"] = omega0.astype(f)
    return out


def _install_loud_hook():
    import traceback

    from concourse import bass2jax

    if getattr(bass2jax, "_loud_hook_installed", False):
        return
    orig = bass2jax.neuronx_cc_hook

    def loud(*a, **k):
        try:
            return orig(*a, **k)
        except BaseException:
            traceback.print_exc()
            raise

    bass2jax.neuronx_cc_hook = loud
    bass2jax._loud_hook_installed = True

    import os

    if os.environ.get("KERNEL_LDW_OPT", "0") == "1":
        from concourse import bass_utils as _bu

        if not getattr(_bu, "_ldw_patch", False):
            _orig_rc = _bu.run_command

            def _rc(cmd, **kw):
                cmd = [c.replace("--enable-ldw-opt=false", "--enable-ldw-opt=true")
                       if isinstance(c, str) else c for c in cmd]
                return _orig_rc(cmd, **kw)

            _bu.run_command = _rc
            _bu._ldw_patch = True


def run(inputs, **spmd_kwargs):
    from concourse.bass_utils import run_bass_kernel_spmd

    _install_loud_hook()

    inputs = {k: np.asarray(v) for k, v in inputs.items()}
    T_run = inputs["x"].shape[1]
    nc = _get_nc(T_run)
    in_map = _prep_inputs(**inputs)
    res = run_bass_kernel_spmd(nc, [in_map] * 8, core_ids=list(range(8)),
                               **spmd_kwargs)
    out = res.results[0]["out"].reshape(B, T_run, J).astype(np.float32)
    return out, res


def kernel(**inputs):
    return run(inputs)[0]


# revision 8
# speedup vs baseline: 1.1290x; 1.1290x over previous
"""Trainium2 Bass kernel for nn_ActivationAndBiophysModel.

2-layer GRU (H=512) + FC + antagonist-muscle biophysics, T=512 steps, B=64.

v2: fp8-e4m3 DoubleRow matmuls. The recurrence is PE-column-streaming
bound; DoubleRow fp8 streams 2 weight columns/cycle and contracts K=256
per pass (vs bf16 1 col/cycle, K=128). Precision scheme (validated in
sim, rel_err 4.9e-3 vs 2e-2 gate):
  - r/z gates: single pass  W_hi(x512) . h_hi(x16)        [S = 8192]
  - n gates + FC: 3 passes  W_hi.h_hi + W_ds(x16).h_lo(x512) + W_lo.h_hi
  - x path + all biases: bf16 plain matmuls (exact); b_hh*n biases ride
    the PSUM-descale scalar_tensor_tensor op.
  - h carried bf16; h_hi/h_lo fp8 stationaries rebuilt per step via PE
    transpose + convert ops.
"""

import sys

for p in ("/opt/trn_rl_repo", "/opt/pypackages"):
    if p not in sys.path:
        sys.path.insert(0, p)

import numpy as np  # noqa: E402

B, T, IN, H, J = 64, 512, 16, 512, 8
HG = 3 * H

SW = 512.0      # W_hi scale
SH = 16.0       # h_hi scale
SL = 512.0      # h_lo scale
SDS = 16.0      # W_ds scale (pairs with h_lo: 16*512 = 8192)
S = SW * SH     # 8192 PSUM scale
C = 1.0 / S

# muscle / joint constants
K0, K1, L0m, L1m, Mm = 100.0, 2000.0, 0.06, 0.006, 0.05
Ij, Kj, Bj, DT = 0.004, 5.0, 0.3, 1.0 / 60.0
_c = DT / Ij
ALPHA = 1.0 - _c * Bj
BETA = _c * Mm * (K0 * L1m + K1 * L0m)
GAMMA = _c * Mm * K1 * L1m
DELTA = _c * (-(2.0 * Mm * Mm * K0) - Kj)
EPS = _c * (-(Mm * Mm * K1))


def _build(nc, bass, tile, mybir, T_run):
    f32 = mybir.dt.float32
    bf = mybir.dt.bfloat16
    f8 = mybir.dt.float8e4
    AF = mybir.ActivationFunctionType
    Alu = mybir.AluOpType
    DR = mybir.MatmulPerfMode.DoubleRow

    # ---- DRAM parameters -------------------------------------------------
    xT_d = nc.declare_dram_parameter("xT", [IN + 1, T_run, B], bf, isOutput=False)
    w0x_d = nc.declare_dram_parameter("w0x", [IN + 1, HG], bf, isOutput=False)
    whi_d = {}
    wn_d = {}
    for nm in ("w0h", "w1i", "w1h"):
        whi_d[nm] = nc.declare_dram_parameter(f"{nm}hi", [4, 128, 1024], f8, isOutput=False)
        wn_d[nm] = nc.declare_dram_parameter(f"{nm}n", [4, 128, H], bf, isOutput=False)
    wfc_d = nc.declare_dram_parameter("wfc", [4, 128, 2 * J], bf, isOutput=False)
    # b1rz*S (1024) | b_ih1n*S (512) | fc_b*S (16) | ones (B)
    brows_d = nc.declare_dram_parameter("brows", [1, 1552 + B], bf, isOutput=False)
    bhn_d = nc.declare_dram_parameter("bhn", [B, 1024], bf, isOutput=False)
    ident_d = nc.declare_dram_parameter("ident", [B, B], bf, isOutput=False)
    hb0_d = nc.declare_dram_parameter("hb0", [2, B, H], bf, isOutput=False)
    hT8hi0_d = nc.declare_dram_parameter("hT8hi0", [2, 128, 4, B], f8, isOutput=False)
    hT0_d = nc.declare_dram_parameter("hT0", [2, 128, 4, B], bf, isOutput=False)
    th0_d = nc.declare_dram_parameter("th0", [B, J], f32, isOutput=False)
    om0_d = nc.declare_dram_parameter("om0", [B, J], f32, isOutput=False)
    out_d = nc.declare_dram_parameter("out", [B, T_run * J], f32, isOutput=True)

    with tile.TileContext(nc) as tc:
        with (
            tc.tile_pool(name="wpool", bufs=1) as wp,
            tc.tile_pool(name="xpool", bufs=8) as xp,
            tc.tile_pool(name="state", bufs=2) as sp,
            tc.tile_pool(name="gates", bufs=2) as gp,
            tc.tile_pool(name="bp", bufs=2) as bp,
            tc.tile_pool(name="prz", bufs=2, space="PSUM") as prz,
            tc.tile_pool(name="psm", bufs=4, space="PSUM") as psm,
        ):
            # ---- load constants/weights once -----------------------------
            w0x = wp.tile([IN + 1, HG], bf)
            nc.sync.dma_start(w0x[:], w0x_d[:])
            whi = {}
            wn = {}
            for nm in ("w0h", "w1i", "w1h"):
                whi[nm] = wp.tile([128, 4, 1024], f8, tag=f"whi{nm}", name=f"whi{nm}")
                wn[nm] = wp.tile([128, 4, H], bf, tag=f"wn{nm}", name=f"wn{nm}")
                for c4 in range(4):
                    nc.sync.dma_start(whi[nm][:, c4, :], whi_d[nm][c4])
                    nc.sync.dma_start(wn[nm][:, c4, :], wn_d[nm][c4])
            wfc = wp.tile([128, 4, 2 * J], bf)
            for c4 in range(4):
                nc.sync.dma_start(wfc[:, c4, :], wfc_d[c4])
            brows = wp.tile([1, 1552 + B], bf)
            nc.sync.dma_start(brows[:], brows_d[:])
            ones = brows[:, 1552 : 1552 + B]
            bhn = wp.tile([B, 1024], bf)
            nc.sync.dma_start(bhn[:], bhn_d[:])
            ident = wp.tile([B, B], bf)
            nc.sync.dma_start(ident[:], ident_d[:])
            out_sb = wp.tile([B, T_run * J], f32)

            # ---- initial state -------------------------------------------
            h0b = sp.tile([B, H], bf, tag="h0b")
            h1b = sp.tile([B, H], bf, tag="h1b")
            nc.sync.dma_start(h0b[:], hb0_d[0])
            nc.sync.dma_start(h1b[:], hb0_d[1])
            h0hi = sp.tile([128, 4, B], f8, tag="h0hi")
            h0T = sp.tile([128, 4, B], bf, tag="h0T")
            h1hi = sp.tile([128, 4, B], f8, tag="h1hi")
            h1T = sp.tile([128, 4, B], bf, tag="h1T")
            for c4 in range(4):
                nc.sync.dma_start(h0hi[:, c4, :], hT8hi0_d[0, :, c4, :])
                nc.sync.dma_start(h0T[:, c4, :], hT0_d[0, :, c4, :])
                nc.sync.dma_start(h1hi[:, c4, :], hT8hi0_d[1, :, c4, :])
                nc.sync.dma_start(h1T[:, c4, :], hT0_d[1, :, c4, :])
            th_init = sp.tile([B, J], f32, tag="th")
            nc.sync.dma_start(th_init[:], th0_d[:])
            om = sp.tile([B, J], f32, tag="om")
            nc.sync.dma_start(om[:], om0_d[:])
            th_ap = th_init[:]

            xts = [None] * (T_run + 2)

            def dma_x(t):
                if t < T_run and xts[t] is None:
                    xt = xp.tile([IN + 1, B], bf, tag="xt")
                    nc.sync.dma_start(xt[:], xT_d[:, t, :])
                    xts[t] = xt

            def mm(out, lhsT, rhs, start, stop):
                nc.tensor.matmul(out, lhsT, rhs, start=start, stop=stop,
                                 perf_mode=DR)

            def mmb(out, lhsT, rhs, start, stop):
                nc.tensor.matmul(out, lhsT, rhs, start=start, stop=stop)

            def l0_mms(rz, hn, inn, hhi, hT, xt):
                w = whi["w0h"]
                # rz first (so sigmoid launches early)
                for p in range(2):
                    st = hhi[:, 2 * p : 2 * p + 2, :]
                    for b in range(2):
                        mm(rz[:, b * 512 : b * 512 + 512], st,
                           w[:, 2 * p : 2 * p + 2, b * 512 : b * 512 + 512],
                           start=(p == 0), stop=False)
                for b in range(2):
                    mmb(rz[:, b * 512 : b * 512 + 512], xt[:],
                        w0x[:, b * 512 : b * 512 + 512], start=False, stop=True)
                for c4 in range(4):
                    mmb(hn[:], hT[:, c4, :], wn["w0h"][:, c4, :],
                        start=(c4 == 0), stop=(c4 == 3))
                mmb(inn[:], xt[:], w0x[:, 1024:1536], start=True, stop=True)

            def l1_hh_mms(rz, hn, hhi, hT):
                w = whi["w1h"]
                for p in range(2):
                    st = hhi[:, 2 * p : 2 * p + 2, :]
                    for b in range(2):
                        mm(rz[:, b * 512 : b * 512 + 512], st,
                           w[:, 2 * p : 2 * p + 2, b * 512 : b * 512 + 512],
                           start=(p == 0), stop=False)
                for c4 in range(4):
                    mmb(hn[:], hT[:, c4, :], wn["w1h"][:, c4, :],
                        start=(c4 == 0), stop=(c4 == 3))

            def l1_ih_mms(rz, inn, hhi, hT):
                w = whi["w1i"]
                for p in range(2):
                    st = hhi[:, 2 * p : 2 * p + 2, :]
                    for b in range(2):
                        mm(rz[:, b * 512 : b * 512 + 512], st,
                           w[:, 2 * p : 2 * p + 2, b * 512 : b * 512 + 512],
                           start=False, stop=False)
                # bf16 bias rows close the rz group early
                for b in range(2):
                    mmb(rz[:, b * 512 : b * 512 + 512], ones,
                        brows[:, b * 512 : b * 512 + 512], start=False, stop=True)
                for c4 in range(4):
                    mmb(inn[:], hT[:, c4, :], wn["w1i"][:, c4, :],
                        start=(c4 == 0), stop=False)
                mmb(inn[:], ones, brows[:, 1024:1536], start=False, stop=True)

            def fc_mms(fc, hT):
                for c4 in range(4):
                    mmb(fc, hT[:, c4, :], wfc[:, c4, :],
                        start=(c4 == 0), stop=False)
                mmb(fc, ones, brows[:, 1536:1552], start=False, stop=True)

            def gru_vec(rz, hn, inn, lo, hb_prev, hb_tag):
                rzs = gp.tile([B, 1024], bf, tag=f"rzs{lo}")
                nc.scalar.activation(rzs[:], rz[:], AF.Sigmoid, scale=C)
                r_ = rzs[:, 0:512]
                z_ = rzs[:, 512:1024]
                hnb = gp.tile([B, H], bf, tag=f"hnb{lo}")
                nc.vector.scalar_tensor_tensor(
                    hnb[:], hn[:], C, bhn[:, lo * 512 : lo * 512 + 512],
                    Alu.mult, Alu.add)
                t_ = gp.tile([B, H], bf, tag=f"t{lo}")
                nc.vector.tensor_mul(t_[:], r_, hnb[:])
                u_ = gp.tile([B, H], bf, tag=f"u{lo}")
                nc.vector.scalar_tensor_tensor(
                    u_[:], inn[:], C, t_[:], Alu.mult, Alu.add)
                n_ = gp.tile([B, H], bf, tag=f"n{lo}")
                nc.scalar.activation(n_[:], u_[:], AF.Tanh)
                d_ = gp.tile([B, H], bf, tag=f"d{lo}")
                nc.vector.tensor_sub(d_[:], hb_prev[:], n_[:])
                m_ = gp.tile([B, H], bf, tag=f"m{lo}")
                nc.vector.tensor_mul(m_[:], z_, d_[:])
                hb = sp.tile([B, H], bf, tag=hb_tag)
                nc.vector.tensor_add(hb[:], m_[:], n_[:])
                return hb

            def transpose_convert(hb, hi_tag, hT_tag, lo):
                ph_a = psm.tile([128, 2, B], bf, tag="sm")
                ph_b = psm.tile([128, 2, B], bf, tag="sm")
                hT = sp.tile([128, 4, B], bf, tag=hT_tag)
                for c4 in range(2):
                    nc.tensor.transpose(ph_a[:, c4, :],
                                        hb[:, c4 * 128 : (c4 + 1) * 128], ident[:])
                nc.vector.tensor_copy(hT[:, 0:2, :], ph_a[:])
                for c4 in range(2, 4):
                    nc.tensor.transpose(ph_b[:, c4 - 2, :],
                                        hb[:, c4 * 128 : (c4 + 1) * 128], ident[:])
                nc.scalar.activation(hT[:, 2:4, :], ph_b[:], AF.Copy)
                hi = sp.tile([128, 4, B], f8, tag=hi_tag)
                nc.scalar.activation(hi[:], hT[:], AF.Copy, scale=SH)
                return hi, hT

            # ---- prologue ------------------------------------------------
            dma_x(0)
            dma_x(1)
            rz0 = prz.tile([B, 1024], f32, tag="rz")
            hn0 = psm.tile([B, 512], f32, tag="sm")
            inn0 = psm.tile([B, 512], f32, tag="sm")
            l0_mms(rz0, hn0, inn0, h0hi, h0T, xts[0])
            rz1 = prz.tile([B, 1024], f32, tag="rz")
            hn1 = psm.tile([B, 512], f32, tag="sm")
            l1_hh_mms(rz1, hn1, h1hi, h1T)

            # ---- time loop -----------------------------------------------
            for t in range(T_run):
                dma_x(t + 2)
                last = t + 1 >= T_run

                h0b = gru_vec(rz0, hn0, inn0, 0, h0b, "h0b")
                h0hi, h0T = transpose_convert(h0b, "h0hi", "h0T", 0)

                inn1 = psm.tile([B, 512], f32, tag="sm")
                l1_ih_mms(rz1, inn1, h0hi, h0T)
                if not last:
                    rz0_n = prz.tile([B, 1024], f32, tag="rz")
                    hn0 = psm.tile([B, 512], f32, tag="sm")
                    inn0 = psm.tile([B, 512], f32, tag="sm")
                    l0_mms(rz0_n, hn0, inn0, h0hi, h0T, xts[t + 1])

                h1b = gru_vec(rz1, hn1, inn1, 1, h1b, "h1b")
                h1hi, h1T = transpose_convert(h1b, "h1hi", "h1T", 1)

                ps_fc = psm.tile([B, 2 * J], f32, tag="sm")
                fc_mms(ps_fc[:, :], h1T)
                if not last:
                    rz1_n = prz.tile([B, 1024], f32, tag="rz")
                    hn1 = psm.tile([B, 512], f32, tag="sm")
                    l1_hh_mms(rz1_n, hn1, h1hi, h1T)
                    rz0, rz1 = rz0_n, rz1_n

                # fc sigmoid + biophysics
                a_s = bp.tile([B, J, 2], f32, tag="as")
                nc.scalar.activation(a_s[:], ps_fc[:, :], AF.Sigmoid, scale=C)
                a0 = a_s[:, :, 0]
                a1 = a_s[:, :, 1]
                s_ = bp.tile([B, J], f32, tag="s")
                nc.gpsimd.tensor_add(s_[:], a1, a0)
                dd = bp.tile([B, J], f32, tag="dd")
                nc.gpsimd.tensor_sub(dd[:], a1, a0)
                p_ = bp.tile([B, J], f32, tag="p")
                nc.gpsimd.tensor_mul(p_[:], s_[:], dd[:])
                w_ = bp.tile([B, J], f32, tag="w")
                nc.vector.scalar_tensor_tensor(
                    w_[:], dd[:], BETA / GAMMA, p_[:], Alu.mult, Alu.add)
                v_ = bp.tile([B, J], f32, tag="v")
                nc.vector.tensor_scalar(v_[:], s_[:], EPS, DELTA,
                                        Alu.mult, Alu.add)
                u2 = bp.tile([B, J], f32, tag="u2")
                nc.vector.tensor_mul(u2[:], v_[:], th_ap)
                q_ = bp.tile([B, J], f32, tag="q")
                nc.vector.scalar_tensor_tensor(
                    q_[:], w_[:], GAMMA, u2[:], Alu.mult, Alu.add)
                om_new = sp.tile([B, J], f32, tag="om")
                nc.vector.scalar_tensor_tensor(
                    om_new[:], om[:], ALPHA, q_[:], Alu.mult, Alu.add)
                om = om_new
                th_new = out_sb[:, t * J : (t + 1) * J]
                nc.vector.scalar_tensor_tensor(
                    th_new, om[:], DT, th_ap, Alu.mult, Alu.add)
                th_ap = th_new

            nc.sync.dma_start(out_d[:], out_sb[:])
    return nc


_NC_CACHE = {}


def _get_nc(T_run):
    if T_run in _NC_CACHE:
        return _NC_CACHE[T_run]
    from concourse import bass, bacc, tile

    mybir = bass.mybir
    nc = bacc.Bacc(None, target_bir_lowering=False)
    _build(nc, bass, tile, mybir, T_run)
    nc.compile()
    _NC_CACHE[T_run] = nc
    return nc


def _prep_inputs(x, W_ih0, W_hh0, b_ih0, b_hh0, W_ih1, W_hh1, b_ih1, b_hh1,
                 fc_W, fc_b, h0, theta0, omega0):
    import ml_dtypes

    F8 = ml_dtypes.float8_e4m3
    BF = ml_dtypes.bfloat16
    T_run = x.shape[1]
    f = np.float32

    def f8q(a, s):
        return np.clip(np.asarray(a, f) * s, -224, 224).astype(F8)

    out = {}

    xT = np.concatenate(
        [np.ascontiguousarray(x.transpose(2, 1, 0)),
         np.ones((1, T_run, B), f)], axis=0).astype(f)
    out["xT"] = xT.astype(BF)

    # layer-0 bias row rides w0x (bf16, pre-scaled by S)
    b0rz = (b_ih0 + b_hh0)[:1024]
    w0x = np.concatenate(
        [W_ih0.T, np.concatenate([b0rz, b_ih0[1024:]])[None, :]], axis=0
    ).astype(f)
    out["w0x"] = (w0x * S).astype(BF)

    for nm, W in (("w0h", W_hh0), ("w1i", W_ih1), ("w1h", W_hh1)):
        WT = np.ascontiguousarray(W.T).astype(f)        # [512, 1536]
        hi = f8q(WT[:, :1024], SW)
        out[f"{nm}hi"] = np.ascontiguousarray(hi.reshape(4, 128, 1024))
        out[f"{nm}n"] = np.ascontiguousarray(
            (WT[:, 1024:] * S).astype(BF).reshape(4, 128, H))

    WfcT = np.ascontiguousarray(fc_W.T).astype(f)        # [512, 16]
    out["wfc"] = np.ascontiguousarray((WfcT * S).astype(BF).reshape(4, 128, 2 * J))

    brows = np.zeros((1, 1552 + B), f)
    brows[0, 0:1024] = (b_ih1 + b_hh1)[:1024] * S
    brows[0, 1024:1536] = b_ih1[1024:] * S
    brows[0, 1536:1552] = fc_b * S
    brows[0, 1552:] = 1.0
    out["brows"] = brows.astype(BF)

    bhn = np.concatenate([np.tile(b_hh0[1024:][None, :], (B, 1)),
                          np.tile(b_hh1[1024:][None, :], (B, 1))],
                         axis=1).astype(f)
    out["bhn"] = bhn.astype(BF)
    out["ident"] = np.eye(B, dtype=f).astype(BF)
    out["hb0"] = h0.astype(f).astype(BF)

    hT = np.stack([np.ascontiguousarray(h0[l].T).astype(f) for l in range(2)])
    hThi = f8q(hT, SH)                                   # [2, 512, 64]
    out["hT8hi0"] = np.ascontiguousarray(
        hThi.reshape(2, 4, 128, B).transpose(0, 2, 1, 3))
    out["hT0"] = np.ascontiguousarray(
        hT.astype(BF).reshape(2, 4, 128, B).transpose(0, 2, 1, 3))
    out["th0"] = theta0.astype(f)
    out["om0"] = omega0.astype(f)
    return out


def _install_loud_hook():
    import traceback

    from concourse import bass2jax

    if getattr(bass2jax, "_loud_hook_installed", False):
        return
    orig = bass2jax.neuronx_cc_hook

    def loud(*a, **k):
        try:
            return orig(*a, **k)
        except BaseException:
            traceback.print_exc()
            raise

    bass2jax.neuronx_cc_hook = loud
    bass2jax._loud_hook_installed = True

    import os

    if os.environ.get("KERNEL_LDW_OPT", "0") == "1":
        from concourse import bass_utils as _bu

        if not getattr(_bu, "_ldw_patch", False):
            _orig_rc = _bu.run_command

            def _rc(cmd, **kw):
                cmd = [c.replace("--enable-ldw-opt=false", "--enable-ldw-opt=true")
                       if isinstance(c, str) else c for c in cmd]
                return _orig_rc(cmd, **kw)

            _bu.run_command = _rc
            _bu._ldw_patch = True


def run(inputs, **spmd_kwargs):
    from concourse.bass_utils import run_bass_kernel_spmd

    _install_loud_hook()

    inputs = {k: np.asarray(v) for k, v in inputs.items()}
    T_run = inputs["x"].shape[1]
    nc = _get_nc(T_run)
    in_map = _prep_inputs(**inputs)
    res = run_bass_kernel_spmd(nc, [in_map] * 8, core_ids=list(range(8)),
                               **spmd_kwargs)
    out = res.results[0]["out"].reshape(B, T_run, J).astype(np.float32)
    return out, res


def kernel(**inputs):
    return run(inputs)[0]
